# revision 1
# baseline (speedup 1.0000x reference)
"""Relation-aware attention (Shaw et al.) for nn_AttentionLayer_532575944777.

Strategy: sequence-parallel shard of the query axis i across 8 NeuronCores.
Each core gets alpha_K[i0:i0+128], alpha_V[i0:i0+128] (32 MB each instead of
256 MB replicated), q[:, i0:i0+128], and full k/v — this is the
memory-roofline-optimal split (~76 MB HBM traffic per core).

Self-contained: hardcodes B=16, S=1024, DIN=D=64, 8 cores.
"""

import numpy as np

B, S, DIN, D = 16, 1024, 64, 64
M = 8          # cores
SL = S // M    # 128 queries per core


def _shard_body(aK, aV, q_sh, k, v, inv_sqrt_d):
    import jax
    import jax.numpy as jnp

    hi = jax.lax.Precision.HIGHEST
    scores = (
        jnp.einsum("bid,bjd->bij", q_sh, k, precision=hi)
        + jnp.einsum("bid,ijd->bij", q_sh, aK, precision=hi)
    ) * inv_sqrt_d
    attn = jax.nn.softmax(scores, axis=-1)
    z = jnp.einsum("bij,bjd->bid", attn, v, precision=hi) + jnp.einsum(
        "bij,ijd->bid", attn, aV, precision=hi
    )
    return z, attn


def _numpy_fallback(x, Wq, Wk, Wv, alpha_K, alpha_V):
    inv = np.float32(1.0 / np.sqrt(D))
    q = x @ Wq
    k = x @ Wk
    v = x @ Wv
    z = np.empty((B, S, D), np.float32)
    attn = np.empty((B, S, S), np.float32)
    for i0 in range(0, S, SL):
        sl = slice(i0, i0 + SL)
        sc = np.einsum("bid,bjd->bij", q[:, sl], k)
        sc += np.einsum("bid,ijd->bij", q[:, sl], alpha_K[sl])
        sc *= inv
        sc -= sc.max(-1, keepdims=True)
        np.exp(sc, out=sc)
        sc /= sc.sum(-1, keepdims=True)
        attn[:, sl] = sc
        z[:, sl] = np.einsum("bij,bjd->bid", sc, v) + np.einsum(
            "bij,ijd->bid", sc, alpha_V[sl]
        )
    return z, attn


def kernel(x, Wq, Wk, Wv, alpha_K, alpha_V):
    x = np.asarray(x, np.float32)
    Wq = np.asarray(Wq, np.float32)
    Wk = np.asarray(Wk, np.float32)
    Wv = np.asarray(Wv, np.float32)
    alpha_K = np.asarray(alpha_K, np.float32)
    alpha_V = np.asarray(alpha_V, np.float32)

    try:
        import jax
        import jax.numpy as jnp
        from functools import partial

        if jax.local_device_count() < M:
            raise RuntimeError("need 8 cores")

        inv = np.float32(1.0 / np.sqrt(D))
        # projections are tiny; do them on host so only the heavy part ships
        q = x @ Wq
        k = x @ Wk
        v = x @ Wv

        q_sh = np.ascontiguousarray(
            q.reshape(B, M, SL, D).transpose(1, 0, 2, 3)
        )  # [M, B, SL, D]
        aK_sh = alpha_K.reshape(M, SL, S, D)
        aV_sh = alpha_V.reshape(M, SL, S, D)

        fn = jax.pmap(
            partial(_shard_body, inv_sqrt_d=inv),
            in_axes=(0, 0, 0, None, None),
        )
        z_sh, attn_sh = fn(aK_sh, aV_sh, q_sh, jnp.asarray(k), jnp.asarray(v))
        z = (
            np.asarray(z_sh)
            .transpose(1, 0, 2, 3)
            .reshape(B, S, D)
            .astype(np.float32)
        )
        attn = (
            np.asarray(attn_sh)
            .transpose(1, 0, 2, 3)
            .reshape(B, S, S)
            .astype(np.float32)
        )
        return z, attn
    except Exception:
        return _numpy_fallback(x, Wq, Wk, Wv, alpha_K, alpha_V)
